# revision 24
# baseline (speedup 1.0000x reference)
import numpy as np

L, B, D, H, K = 3, 32768, 1024, 8, 64
N_CORES = 8
B_LOC = B // N_CORES
P = 128
R = 2
N_TILES = B_LOC // (P * R)
FREE = R * D
XOFF = 64
C1 = XOFF + FREE
DPP = 2176
W2 = D + 2

_cache = {}


def _build_program(zero_cb: bool):
    import concourse.bass as bass
    from concourse import mybir

    F32 = mybir.dt.float32
    BF16 = mybir.dt.bfloat16
    MUL = mybir.AluOpType.mult
    ADD = mybir.AluOpType.add

    nc = bass.Bass()
    x = nc.declare_dram_parameter("x", [N_TILES * P, FREE], BF16, isOutput=False)
    u = nc.declare_dram_parameter("u", [P, D + 4], BF16, isOutput=False)
    cb = nc.declare_dram_parameter("cb", [1, D], F32, isOutput=False)
    out = nc.declare_dram_parameter("out", [N_TILES * P, FREE], BF16, isOutput=True)

    cb_bcast = bass.AP(tensor=cb.ap().tensor, offset=0, ap=[[0, P], [1, D]])

    LAST = N_TILES - 1

    with (
        nc.sbuf_tensor([P, D + 4], BF16) as ub,
        nc.sbuf_tensor([P, D], F32) as cbb,
        nc.sbuf_tensor([P, N_TILES, DPP], BF16) as xt,
        nc.sbuf_tensor([P, N_TILES, R, W2], BF16) as oscr,
        nc.sbuf_tensor([P, N_TILES, R], F32) as tsc,
        nc.sbuf_tensor([P, 1], BF16) as warm,
        nc.semaphore("us") as us,
        nc.semaphore("ld0b") as ld0b,
        nc.semaphore("cm") as cm,
        nc.semaphore("ta") as ta,
        nc.semaphore("cm2") as cm2,
        nc.semaphore("cm3") as cm3,
        nc.semaphore("st2") as st2,
        nc.Block() as block,
    ):
        lds = [nc.alloc_semaphore(f"ld{i}") for i in range(N_TILES)]

        if zero_cb:

            @block.sync
            def _(sync):
                sync.dma_start(out=ub[:, :], in_=u.ap()).then_inc(us, 16)
                for i in range(0, N_TILES):
                    sync.dma_start(
                        out=xt[:, i, XOFF:C1], in_=x[i * P : (i + 1) * P, :]
                    ).then_inc(lds[i], 16)
                store_order = list(range(13)) + [15, 13, 14]
                store_cm3 = {15: 15, 13: 18, 14: 19}
                for i in store_order:
                    if i <= 12:
                        sync.wait_ge(cm2, i + 1)
                        sync.wait_ge(cm3, i + 1)
                    else:
                        sync.wait_ge(cm3, store_cm3[i])
                    sync.dma_start(
                        out=out[i * P : (i + 1) * P, :], in_=xt[:, i, XOFF:C1]
                    ).then_inc(st2, 16)

            @block.vector
            def _(vector):
                nc.vector.memset(xt[:, :, XOFF - 2 : XOFF], 1.0).then_inc(cm, 1)
                nc.vector.memset(xt[:, :, C1 : C1 + 2], 1.0).then_inc(cm, 1)
                vector.wait_ge(us, 16)
                vector.wait_ge(cm, 2)
                for i in range(N_TILES - 1):
                    vector.wait_ge(lds[i], 16)
                    nc.vector.scalar_tensor_tensor(
                        out=oscr[:, i, 0, :],
                        in0=xt[:, i, XOFF - 2 : XOFF - 2 + W2],
                        scalar=1.0,
                        in1=ub[:, 0:W2],
                        op0=MUL,
                        op1=MUL,
                        accum_out=tsc[:, i, 0:1],
                    ).then_inc(cm, 1)
                    nc.vector.tensor_tensor(
                        out=oscr[:, i, 1, :],
                        in0=xt[:, i, XOFF - 2 + W2 : XOFF - 2 + 2 * W2],
                        in1=ub[:, 2 : 2 + W2],
                        op=MUL,
                    ).then_inc(cm, 1)
                    if i >= 2:
                        j = i - 2
                        vector.wait_ge(ta, j + 1)
                        nc.vector.tensor_scalar_mul(
                            out=xt[:, j, XOFF + D : XOFF + 2 * D],
                            in0=xt[:, j, XOFF + D : XOFF + 2 * D],
                            scalar1=tsc[:, j, 1:2],
                        ).then_inc(cm3, 1)
                vector.wait_ge(lds[LAST], 16)
                for r in range(R):
                    nc.vector.scalar_tensor_tensor(
                        out=oscr[:, LAST, r, :],
                        in0=xt[:, LAST, XOFF - 2 + r * W2 : XOFF - 2 + (r + 1) * W2],
                        scalar=1.0,
                        in1=ub[:, 2 * r : 2 * r + W2],
                        op0=MUL,
                        op1=MUL,
                        accum_out=tsc[:, LAST, r : r + 1],
                    ).then_inc(cm, 1)
                vector.wait_ge(cm, 2 + 2 * N_TILES)
                for j, r in ((LAST, 0), (LAST, 1), (13, 0), (14, 0)):
                    nc.vector.tensor_scalar_mul(
                        out=xt[:, j, XOFF + r * D : XOFF + (r + 1) * D],
                        in0=xt[:, j, XOFF + r * D : XOFF + (r + 1) * D],
                        scalar1=tsc[:, j, r : r + 1],
                    ).then_inc(cm3, 1)
                vector.wait_ge(ta, 14)
                nc.vector.tensor_scalar_mul(
                    out=xt[:, 13, XOFF + D : XOFF + 2 * D],
                    in0=xt[:, 13, XOFF + D : XOFF + 2 * D],
                    scalar1=tsc[:, 13, 1:2],
                ).then_inc(cm3, 1)
                vector.wait_ge(ta, 15)
                nc.vector.tensor_scalar_mul(
                    out=xt[:, 14, XOFF + D : XOFF + 2 * D],
                    in0=xt[:, 14, XOFF + D : XOFF + 2 * D],
                    scalar1=tsc[:, 14, 1:2],
                ).then_inc(cm3, 1)

            @block.scalar
            def _(scalar):
                scalar.wait_ge(us, 16)
                nc.scalar.mul(out=warm[:, :], in_=ub[:, 0:1], mul=1.0)
                for i in range(N_TILES - 1):
                    scalar.wait_ge(cm, 2 * i + 4)
                    nc.scalar.activation(
                        out=oscr[:, i, 1, :],
                        in_=oscr[:, i, 1, :],
                        func=mybir.ActivationFunctionType.Copy,
                        scale=1.0,
                        accum_out=tsc[:, i, 1:2],
                    ).then_inc(ta, 1)
                    if i <= 12:
                        nc.scalar.mul(
                            out=xt[:, i, XOFF : XOFF + D],
                            in_=xt[:, i, XOFF : XOFF + D],
                            mul=tsc[:, i, 0:1],
                        ).then_inc(cm2, 1)

        else:
            st = nc.alloc_semaphore("st")
            u_bcast = u.ap()

            @block.sync
            def _(sync):
                ev = bass.AP(tensor=x.ap().tensor, offset=0, ap=[[2 * D, P], [1, D]])
                od = bass.AP(tensor=x.ap().tensor, offset=D, ap=[[2 * D, P], [1, D]])
                sync.dma_start(out=xt[:, 0, XOFF : XOFF + D], in_=ev).then_inc(lds[0], 16)
                sync.dma_start(out=xt[:, 0, XOFF + D : C1], in_=od).then_inc(ld0b, 16)
                for i in range(1, N_TILES):
                    sync.dma_start(
                        out=xt[:, i, XOFF:C1], in_=x[i * P : (i + 1) * P, :]
                    ).then_inc(lds[i], 16)

            @block.vector
            def _(vector):
                nc.vector.memset(xt[:, :, XOFF - 2 : XOFF], 1.0).then_inc(cm, 1)
                nc.vector.memset(xt[:, :, C1 : C1 + 2], 1.0).then_inc(cm, 1)
                vector.wait_ge(us, 32)
                vector.wait_ge(cm, 2)
                for i in range(N_TILES):
                    vector.wait_ge(lds[i], 16)
                    for r in range(R):
                        if i == 0 and r == 1:
                            vector.wait_ge(ld0b, 16)
                        nc.vector.scalar_tensor_tensor(
                            out=oscr[:, i, r, :],
                            in0=xt[:, i, XOFF - 2 + r * W2 : XOFF - 2 + (r + 1) * W2],
                            scalar=1.0,
                            in1=ub[:, 2 * r : 2 * r + W2],
                            op0=MUL,
                            op1=MUL,
                            accum_out=tsc[:, i, r : r + 1],
                        ).then_inc(cm, 1)
                    vector.wait_ge(cm, 2 + R * (i + 1))
                    for r in range(R):
                        nc.vector.scalar_tensor_tensor(
                            out=xt[:, i, XOFF + r * D : XOFF + (r + 1) * D],
                            in0=xt[:, i, XOFF + r * D : XOFF + (r + 1) * D],
                            scalar=tsc[:, i, r : r + 1],
                            in1=cbb[:, :],
                            op0=MUL,
                            op1=ADD,
                        ).then_inc(cm2, 1)

            @block.gpsimd
            def _(gpsimd):
                gpsimd.dma_start(out=ub[:, :], in_=u_bcast).then_inc(us, 16)
                gpsimd.dma_start(out=cbb[:, :], in_=cb_bcast).then_inc(us, 16)
                for i in range(N_TILES):
                    gpsimd.wait_ge(cm2, R * (i + 1))
                    gpsimd.dma_start(
                        out=out[i * P : (i + 1) * P, :], in_=xt[:, i, XOFF:C1]
                    ).then_inc(st, 16)
                gpsimd.wait_ge(st, 16 * N_TILES)

    return nc


def _precompute(wv, bv, wo, bo, cw, cb):
    usum = np.zeros(D, np.float64)
    cprime = 1.0
    for i in range(L):
        Wv = wv[i].reshape(D, H * K).astype(np.float64)
        Wo = wo[i].reshape(H * K, D).astype(np.float64)
        cwi = cw[i].reshape(D).astype(np.float64)
        wocw = Wo @ cwi
        usum += Wv @ wocw
        cprime += float(bv[i].reshape(H * K).astype(np.float64) @ wocw)
        cprime += float(bo[i].astype(np.float64) @ cwi)
    cbsum = cb.astype(np.float64).sum(axis=0)
    return usum.astype(np.float32), float(cprime), cbsum.astype(np.float32)


def _ensure_trace_hook_importable():
    try:
        import antenv.axon_hooks
    except ImportError:
        import sys
        import types

        mod = types.ModuleType("antenv.axon_hooks")
        mod.get_axon_ntff_profile_hook = lambda: None
        mod.set_axon_ntff_profile_hook = lambda hook: None
        sys.modules["antenv.axon_hooks"] = mod


def kernel(x, wq, bq, wk, bk, wv, bv, wo, bo, cw, cb):
    import ml_dtypes

    from concourse.bass_utils import run_bass_kernel_spmd

    _ensure_trace_hook_importable()

    bf16 = np.dtype(ml_dtypes.bfloat16)
    x = np.ascontiguousarray(np.asarray(x, dtype=np.float32)).astype(bf16)
    usum, cprime, cbsum = _precompute(
        np.asarray(wv), np.asarray(bv), np.asarray(wo), np.asarray(bo),
        np.asarray(cw), np.asarray(cb),
    )
    zero_cb = not np.any(cbsum)

    if zero_cb not in _cache:
        _cache[zero_cb] = _build_program(zero_cb)
    nc = _cache[zero_cb]

    cA = np.float32(cprime).astype(bf16)
    cB = np.float32(cprime - float(cA)).astype(bf16)
    u2 = np.concatenate(
        [[cA, cB], usum.astype(bf16), [cA, cB]]
    ).astype(bf16).reshape(1, D + 4)
    u2 = np.ascontiguousarray(np.broadcast_to(u2, (P, D + 4)))
    cb2 = cbsum.reshape(1, D)
    in_maps = [
        {
            "x": x[c * B_LOC : (c + 1) * B_LOC].reshape(N_TILES * P, FREE),
            "u": u2,
            "cb": cb2,
        }
        for c in range(N_CORES)
    ]
    res = run_bass_kernel_spmd(nc, in_maps, list(range(N_CORES)))
    out16 = np.concatenate(
        [res.results[c]["out"].reshape(B_LOC, D) for c in range(N_CORES)], axis=0
    )
    return out16.astype(np.float32)


# revision 25
# speedup vs baseline: 1.0029x; 1.0029x over previous
import numpy as np

L, B, D, H, K = 3, 32768, 1024, 8, 64
N_CORES = 8
B_LOC = B // N_CORES
P = 128
R = 2
N_TILES = B_LOC // (P * R)
FREE = R * D
XOFF = 64
C1 = XOFF + FREE
DPP = 2176
W2 = D + 2

_cache = {}


def _build_program(zero_cb: bool):
    import concourse.bass as bass
    from concourse import mybir

    F32 = mybir.dt.float32
    BF16 = mybir.dt.bfloat16
    MUL = mybir.AluOpType.mult
    ADD = mybir.AluOpType.add

    nc = bass.Bass()
    x = nc.declare_dram_parameter("x", [N_TILES * P, FREE], BF16, isOutput=False)
    u = nc.declare_dram_parameter("u", [P, 2 * D + 4], BF16, isOutput=False)
    cb = nc.declare_dram_parameter("cb", [1, D], F32, isOutput=False)
    out = nc.declare_dram_parameter("out", [N_TILES * P, FREE], BF16, isOutput=True)

    cb_bcast = bass.AP(tensor=cb.ap().tensor, offset=0, ap=[[0, P], [1, D]])

    LAST = N_TILES - 1

    with (
        nc.sbuf_tensor([P, D + 4], BF16) as ub,
        nc.sbuf_tensor([P, D], F32) as cbb,
        nc.sbuf_tensor([P, N_TILES, DPP], BF16) as xt,
        nc.sbuf_tensor([P, N_TILES, R, W2], BF16) as oscr,
        nc.sbuf_tensor([P, N_TILES, R], F32) as tsc,
        nc.sbuf_tensor([P, 1], BF16) as warm,
        nc.semaphore("us") as us,
        nc.semaphore("ld0b") as ld0b,
        nc.semaphore("cm") as cm,
        nc.semaphore("ta") as ta,
        nc.semaphore("cm2") as cm2,
        nc.semaphore("cm3") as cm3,
        nc.semaphore("st2") as st2,
        nc.Block() as block,
    ):
        lds = [nc.alloc_semaphore(f"ld{i}") for i in range(N_TILES)]

        if zero_cb:

            @block.sync
            def _(sync):
                sync.dma_start(out=ub[:, :], in_=u[:, 0 : D + 4]).then_inc(us, 16)
                for i in range(0, N_TILES):
                    sync.dma_start(
                        out=xt[:, i, XOFF:C1], in_=x[i * P : (i + 1) * P, :]
                    ).then_inc(lds[i], 16)
                store_order = list(range(13)) + [15, 13, 14]
                store_cm3 = {15: 15, 13: 18, 14: 19}
                for i in store_order:
                    if i <= 12:
                        sync.wait_ge(cm2, i + 1)
                        sync.wait_ge(cm3, i + 1)
                    else:
                        sync.wait_ge(cm3, store_cm3[i])
                    sync.dma_start(
                        out=out[i * P : (i + 1) * P, :], in_=xt[:, i, XOFF:C1]
                    ).then_inc(st2, 16)

            @block.vector
            def _(vector):
                nc.vector.memset(xt[:, :, XOFF - 2 : XOFF], 1.0).then_inc(cm, 1)
                nc.vector.memset(xt[:, :, C1 : C1 + 2], 1.0).then_inc(cm, 1)
                vector.wait_ge(us, 16)
                vector.wait_ge(cm, 2)
                for i in range(N_TILES - 1):
                    vector.wait_ge(lds[i], 16)
                    nc.vector.scalar_tensor_tensor(
                        out=oscr[:, i, 0, :],
                        in0=xt[:, i, XOFF - 2 : XOFF - 2 + W2],
                        scalar=1.0,
                        in1=ub[:, 0:W2],
                        op0=MUL,
                        op1=MUL,
                        accum_out=tsc[:, i, 0:1],
                    ).then_inc(cm, 1)
                    nc.vector.tensor_tensor(
                        out=oscr[:, i, 1, :],
                        in0=xt[:, i, XOFF - 2 + W2 : XOFF - 2 + 2 * W2],
                        in1=ub[:, 2 : 2 + W2],
                        op=MUL,
                    ).then_inc(cm, 1)
                    if i >= 2:
                        j = i - 2
                        vector.wait_ge(ta, j + 1)
                        nc.vector.tensor_scalar_mul(
                            out=xt[:, j, XOFF + D : XOFF + 2 * D],
                            in0=xt[:, j, XOFF + D : XOFF + 2 * D],
                            scalar1=tsc[:, j, 1:2],
                        ).then_inc(cm3, 1)
                vector.wait_ge(lds[LAST], 16)
                for r in range(R):
                    nc.vector.scalar_tensor_tensor(
                        out=oscr[:, LAST, r, :],
                        in0=xt[:, LAST, XOFF - 2 + r * W2 : XOFF - 2 + (r + 1) * W2],
                        scalar=1.0,
                        in1=ub[:, 2 * r : 2 * r + W2],
                        op0=MUL,
                        op1=MUL,
                        accum_out=tsc[:, LAST, r : r + 1],
                    ).then_inc(cm, 1)
                vector.wait_ge(cm, 2 + 2 * N_TILES)
                for j, r in ((LAST, 0), (LAST, 1), (13, 0), (14, 0)):
                    nc.vector.tensor_scalar_mul(
                        out=xt[:, j, XOFF + r * D : XOFF + (r + 1) * D],
                        in0=xt[:, j, XOFF + r * D : XOFF + (r + 1) * D],
                        scalar1=tsc[:, j, r : r + 1],
                    ).then_inc(cm3, 1)
                vector.wait_ge(ta, 14)
                nc.vector.tensor_scalar_mul(
                    out=xt[:, 13, XOFF + D : XOFF + 2 * D],
                    in0=xt[:, 13, XOFF + D : XOFF + 2 * D],
                    scalar1=tsc[:, 13, 1:2],
                ).then_inc(cm3, 1)
                vector.wait_ge(ta, 15)
                nc.vector.tensor_scalar_mul(
                    out=xt[:, 14, XOFF + D : XOFF + 2 * D],
                    in0=xt[:, 14, XOFF + D : XOFF + 2 * D],
                    scalar1=tsc[:, 14, 1:2],
                ).then_inc(cm3, 1)

            @block.scalar
            def _(scalar):
                scalar.wait_ge(us, 16)
                nc.scalar.mul(out=warm[:, :], in_=ub[:, 0:1], mul=1.0)
                for i in range(N_TILES - 1):
                    scalar.wait_ge(cm, 2 * i + 4)
                    nc.scalar.activation(
                        out=oscr[:, i, 1, :],
                        in_=oscr[:, i, 1, :],
                        func=mybir.ActivationFunctionType.Copy,
                        scale=1.0,
                        accum_out=tsc[:, i, 1:2],
                    ).then_inc(ta, 1)
                    if i <= 12:
                        nc.scalar.mul(
                            out=xt[:, i, XOFF : XOFF + D],
                            in_=xt[:, i, XOFF : XOFF + D],
                            mul=tsc[:, i, 0:1],
                        ).then_inc(cm2, 1)

        else:
            st = nc.alloc_semaphore("st")
            u_bcast = u[:, 0 : D + 4]

            @block.sync
            def _(sync):
                ev = bass.AP(tensor=x.ap().tensor, offset=0, ap=[[2 * D, P], [1, D]])
                od = bass.AP(tensor=x.ap().tensor, offset=D, ap=[[2 * D, P], [1, D]])
                sync.dma_start(out=xt[:, 0, XOFF : XOFF + D], in_=ev).then_inc(lds[0], 16)
                sync.dma_start(out=xt[:, 0, XOFF + D : C1], in_=od).then_inc(ld0b, 16)
                for i in range(1, N_TILES):
                    sync.dma_start(
                        out=xt[:, i, XOFF:C1], in_=x[i * P : (i + 1) * P, :]
                    ).then_inc(lds[i], 16)

            @block.vector
            def _(vector):
                nc.vector.memset(xt[:, :, XOFF - 2 : XOFF], 1.0).then_inc(cm, 1)
                nc.vector.memset(xt[:, :, C1 : C1 + 2], 1.0).then_inc(cm, 1)
                vector.wait_ge(us, 32)
                vector.wait_ge(cm, 2)
                for i in range(N_TILES):
                    vector.wait_ge(lds[i], 16)
                    for r in range(R):
                        if i == 0 and r == 1:
                            vector.wait_ge(ld0b, 16)
                        nc.vector.scalar_tensor_tensor(
                            out=oscr[:, i, r, :],
                            in0=xt[:, i, XOFF - 2 + r * W2 : XOFF - 2 + (r + 1) * W2],
                            scalar=1.0,
                            in1=ub[:, 2 * r : 2 * r + W2],
                            op0=MUL,
                            op1=MUL,
                            accum_out=tsc[:, i, r : r + 1],
                        ).then_inc(cm, 1)
                    vector.wait_ge(cm, 2 + R * (i + 1))
                    for r in range(R):
                        nc.vector.scalar_tensor_tensor(
                            out=xt[:, i, XOFF + r * D : XOFF + (r + 1) * D],
                            in0=xt[:, i, XOFF + r * D : XOFF + (r + 1) * D],
                            scalar=tsc[:, i, r : r + 1],
                            in1=cbb[:, :],
                            op0=MUL,
                            op1=ADD,
                        ).then_inc(cm2, 1)

            @block.gpsimd
            def _(gpsimd):
                gpsimd.dma_start(out=ub[:, :], in_=u_bcast).then_inc(us, 16)
                gpsimd.dma_start(out=cbb[:, :], in_=cb_bcast).then_inc(us, 16)
                for i in range(N_TILES):
                    gpsimd.wait_ge(cm2, R * (i + 1))
                    gpsimd.dma_start(
                        out=out[i * P : (i + 1) * P, :], in_=xt[:, i, XOFF:C1]
                    ).then_inc(st, 16)
                gpsimd.wait_ge(st, 16 * N_TILES)

    return nc


def _precompute(wv, bv, wo, bo, cw, cb):
    usum = np.zeros(D, np.float64)
    cprime = 1.0
    for i in range(L):
        Wv = wv[i].reshape(D, H * K).astype(np.float64)
        Wo = wo[i].reshape(H * K, D).astype(np.float64)
        cwi = cw[i].reshape(D).astype(np.float64)
        wocw = Wo @ cwi
        usum += Wv @ wocw
        cprime += float(bv[i].reshape(H * K).astype(np.float64) @ wocw)
        cprime += float(bo[i].astype(np.float64) @ cwi)
    cbsum = cb.astype(np.float64).sum(axis=0)
    return usum.astype(np.float32), float(cprime), cbsum.astype(np.float32)


def _ensure_trace_hook_importable():
    try:
        import antenv.axon_hooks
    except ImportError:
        import sys
        import types

        mod = types.ModuleType("antenv.axon_hooks")
        mod.get_axon_ntff_profile_hook = lambda: None
        mod.set_axon_ntff_profile_hook = lambda hook: None
        sys.modules["antenv.axon_hooks"] = mod


def kernel(x, wq, bq, wk, bk, wv, bv, wo, bo, cw, cb):
    import ml_dtypes

    from concourse.bass_utils import run_bass_kernel_spmd

    _ensure_trace_hook_importable()

    bf16 = np.dtype(ml_dtypes.bfloat16)
    x = np.ascontiguousarray(np.asarray(x, dtype=np.float32)).astype(bf16)
    usum, cprime, cbsum = _precompute(
        np.asarray(wv), np.asarray(bv), np.asarray(wo), np.asarray(bo),
        np.asarray(cw), np.asarray(cb),
    )
    zero_cb = not np.any(cbsum)

    if zero_cb not in _cache:
        _cache[zero_cb] = _build_program(zero_cb)
    nc = _cache[zero_cb]

    cA = np.float32(cprime).astype(bf16)
    cB = np.float32(cprime - float(cA)).astype(bf16)
    u2 = np.concatenate(
        [[cA, cB], usum.astype(bf16), [cA, cB]]
    ).astype(bf16).reshape(1, D + 4)
    u2 = np.concatenate([u2, np.zeros((1, D), bf16)], axis=1)
    u2 = np.ascontiguousarray(np.broadcast_to(u2, (P, 2 * D + 4)))
    cb2 = cbsum.reshape(1, D)
    in_maps = [
        {
            "x": x[c * B_LOC : (c + 1) * B_LOC].reshape(N_TILES * P, FREE),
            "u": u2,
            "cb": cb2,
        }
        for c in range(N_CORES)
    ]
    res = run_bass_kernel_spmd(nc, in_maps, list(range(N_CORES)))
    out16 = np.concatenate(
        [res.results[c]["out"].reshape(B_LOC, D) for c in range(N_CORES)], axis=0
    )
    return out16.astype(np.float32)
